# revision 51
# baseline (speedup 1.0000x reference)
"""Causal attention head kernel for Trainium2, 8 NeuronCores.

Problem: B=4, S=4096, D_IN=512, D_OUT=64, f32, causal, scale=1/sqrt(S).

Sharding: core c -> (batch b = c//2, k-shard hk = c%2). Each core handles ALL
queries of its batch but only the k-tiles (of 128 rows) with tile_index % 2 ==
hk, producing partial (numerator | denominator) sums; the host combines the
two k-shards. SPMD: causality differences between the two k-shards live in a
per-core mask input and in the per-core gather of X_k/X_v rows.

fp16 datapath (inputs cast on host: halves DMA, ~10-bit mantissa keeps the
f32r-class accuracy). q-chunk-outer attention with PV accumulated in PSUM
across all k-pairs of the chunk; softmax denominator rides as a ones-column
(65th row) of V. PE stream is software-pipelined (scores of the next pair are
emitted before PV of the previous one) so exp latency stays off the PE
critical path; warm-up matmuls at t=0 ramp the PE clock before real work
arrives. The diagonal pair's second k-tile only covers the upper q-half
(causally valid for both shards); the diag pair runs last per chunk (its V
tiles are the freshest load) except in the final chunk where it runs first
to keep exp+mask off the tail, and the final pair is split into single-tile
halves. Loads are 512-col chunks, need-order sorted, issued from the SP /
Activation / Pool sequencers (a DMA holds its issuing SEQ for ~2.2us fixed
overhead plus the transfer, and all transfers serialize on the DMA engines);
output staging goes PSUM -> SBUF on DVE (gpsimd cannot access PSUM), with
three batched output DMAs on SP.
"""

import os

os.environ.setdefault("JAX_PLATFORMS", "cpu")

import numpy as np

import concourse.bass as bass
import concourse.bacc as bacc
import concourse.mybir as mybir
from concourse import tile
from concourse.bass_utils import run_bass_kernel_spmd

F32 = mybir.dt.float32
F16 = mybir.dt.float16

B, S, D_IN, D_OUT = 4, 4096, 512, 64
SK = S // 2          # per-core k rows (interleaved 128-tiles)
N_KT = SK // 128     # 16 local k-tiles
N_QC = S // 512      # 8 q-chunks of 512
N_CORES = 8

_CACHE = {}


def mm(nc, out, lhsT, rhs, start, stop):
    nc.tensor.matmul(out, lhsT, rhs, start=start, stop=stop)


def build_nc():
    nc = bacc.Bacc(trn_type="TRN2", target_bir_lowering=False, debug=False)

    xqt_d = nc.dram_tensor("xqt", [D_IN, S], F16, kind="ExternalInput").ap()
    xkt_d = nc.dram_tensor("xkt", [D_IN, SK], F16, kind="ExternalInput").ap()
    xvt_d = nc.dram_tensor("xvt", [D_IN, SK], F16, kind="ExternalInput").ap()
    wts_d = nc.dram_tensor("wts", [128, 768], F16, kind="ExternalInput").ap()
    cm_d = nc.dram_tensor("cmask", [128, 768], F16, kind="ExternalInput").ap()
    pvt = nc.dram_tensor("pvt", [D_OUT + 1, S], F16, kind="ExternalOutput").ap()

    with tile.TileContext(nc) as tc:
        with (
            tc.tile_pool(name="persist", bufs=1) as pp,
            tc.tile_pool(name="et", bufs=5) as etp,
            tc.tile_pool(name="ps_s", bufs=2, space="PSUM") as ps_s,
            tc.tile_pool(name="ps_pv", bufs=2, space="PSUM") as ps_pv,
            tc.tile_pool(name="ps_pr", bufs=2, space="PSUM") as ps_pr,
        ):
            # ---- persistent SBUF tiles ----
            xqT = pp.tile([128, 4, S], F16, tag="xqT", name="xqT")
            xkT = pp.tile([128, 4, SK], F16, tag="xkT", name="xkT")
            xvT = pp.tile([128, 4, SK], F16, tag="xvT", name="xvT")
            qt = pp.tile([64, S], F16, tag="qt", name="qt")
            kt = pp.tile([64, SK], F16, tag="kt", name="kt")
            vaug = pp.tile([128, N_KT, D_OUT + 1], F16, tag="vaug", name="vaug")
            wts = pp.tile([128, 768], F16, tag="wts", name="wts")
            cmask = pp.tile([128, 768], F16, tag="cmask", name="cmask")
            warm = pp.tile([128, 512], F16, tag="warm", name="warm")
            out_sb = pp.tile([D_OUT + 1, S], F16, tag="out_sb", name="out_sb")

            # PE warm-up: ramp the tensor clock while loads are in flight.
            # The values feed an unread PSUM; only the busy time matters.
            # The prelude reads the (not-yet-loaded) cmask tile so the PE can
            # start at t~0 with no memset ahead of it; the cmask DMA simply
            # picks up a WAR dependency on the prelude. Fillers use the
            # memset warm tile (their WAR would otherwise delay the mask).
            wps = ps_s.tile([128, 256], F32, tag="ps_s", name="warm_ps")

            def warmN(n, src_tile=None):
                t = warm if src_tile is None else src_tile
                for _ in range(n):
                    mm(nc, wps[:], t[:, 0:128], t[:, 0:256],
                       start=True, stop=True)

            warmN(8, cmask)
            nc.vector.memset(warm[:, 0:256], 0.0)
            warmN(11)
            w_sb = {nm: wts[:, 256 * i:256 * (i + 1)].rearrange(
                        "p (dt e) -> p dt e", dt=4)
                    for i, nm in enumerate(("wq", "wk", "wv"))}
            # ones column -> softmax denominator via the PV matmul
            nc.vector.memset(vaug[:, :, D_OUT], 1.0)

            def load_cols(eng, dst_tile, src_ap, c0, c1):
                sl = slice(c0, c1)
                src = src_ap.rearrange("(db p) c -> p db c", p=128)
                eng.dma_start(out=dst_tile[:, :, sl], in_=src[:, :, sl])

            def proj_chunk(dst, xT, w, c, nm):
                sl = slice(c * 512, (c + 1) * 512)
                ps = ps_pr.tile([128, 512], F32, tag="ps_pr", name=f"pp_{nm}{c}")
                for dt in range(4):
                    mm(nc, ps[0:64, :], w[:, dt, :], xT[:, dt, sl],
                       start=(dt == 0), stop=(dt == 3))
                nc.vector.tensor_copy(dst[:, sl], ps[0:64, :])

            def v_pair(i):
                # V natural for s-tiles 2i, 2i+1: [128s,64] per tile
                for t in (2 * i, 2 * i + 1):
                    ps = ps_pr.tile([128, 512], F32, tag="ps_pr", name=f"pv_{t}")
                    for dt in range(4):
                        mm(nc, ps[:, 0:D_OUT],
                           xvT[:, dt, t * 128:(t + 1) * 128],
                           w_sb["wv"][:, dt, :],
                           start=(dt == 0), stop=(dt == 3))
                    nc.vector.tensor_copy(vaug[:, t, 0:D_OUT], ps[:, 0:D_OUT])

            # deferred PV pairs: flushed after the next PE work is emitted so
            # the PE never stalls waiting for the exp feeding it
            pend = []

            def flush_pend():
                while pend:
                    pend.pop(0)()

            def attn_chunk(j):
                # diagonal pair last (its V tiles are the freshest load)
                # except in the final chunk, where diag-first keeps the
                # exp+mask chain off the critical tail; PV accumulates in one
                # PSUM tile across all pairs
                qs = qt[:, j * 512:(j + 1) * 512]
                pv = ps_pv.tile([65, 512], F32, tag="ps_pv", name=f"pvp{j}")
                order = [j] + list(range(j)) if j == 7 else \
                    list(range(j)) + [j]
                for n, i in enumerate(order):
                    first, last = (n == 0), (n == len(order) - 1)
                    if i == j:
                        # diag pair: tile 2j over all q, tile 2j+1 only the
                        # upper q-half (lower half fully masked on both
                        # shards); mask input covers the 768 kept columns
                        ps = ps_d = ps_s.tile([128, 768], F32, tag="ps_s",
                                              name=f"st{j}_d")
                        mm(nc, ps[:, 0:512], kt[:, 2 * j * 128:(2 * j + 1) * 128],
                           qs, start=True, stop=True)
                        mm(nc, ps[:, 512:768],
                           kt[:, (2 * j + 1) * 128:(2 * j + 2) * 128],
                           qs[:, 256:512], start=True, stop=True)
                        et = etp.tile([128, 768], F16, tag="et", name=f"et{j}_d")
                        nc.scalar.activation(
                            et[:], ps[:], mybir.ActivationFunctionType.Exp)
                        nc.vector.tensor_mul(et[:], et[:], cmask[:])

                        def pv_pair(i=i, et=et, first=first, last=last):
                            mm(nc, pv[:], vaug[:, 2 * i, :], et[:, 0:512],
                               start=first, stop=False)
                            mm(nc, pv[:, 256:512], vaug[:, 2 * i + 1, :],
                               et[:, 512:768], start=False, stop=last)
                    elif j == 7 and last:
                        # final pair of the final chunk: two single-tile
                        # halves so the last exp covers only 512 columns
                        for h in range(2):
                            t = 2 * i + h
                            ps = ps_s.tile([128, 512], F32, tag="ps_s",
                                           name=f"st7_{i}{h}")
                            mm(nc, ps[:], kt[:, t * 128:(t + 1) * 128], qs,
                               start=True, stop=True)
                            et = etp.tile([128, 512], F16, tag="et",
                                          name=f"et7_{i}{h}")
                            nc.scalar.activation(
                                et[:], ps[:],
                                mybir.ActivationFunctionType.Exp)

                            def pv_half(t=t, et=et, h=h):
                                mm(nc, pv[:], vaug[:, t, :], et[:],
                                   start=False, stop=(h == 1))
                            pend.append(pv_half)
                            if len(pend) > 2:
                                pend.pop(0)()
                        continue
                    else:
                        ps = ps_s.tile([128, 1024], F32, tag="ps_s",
                                       name=f"st{j}_{i}")
                        for h in range(2):
                            t = 2 * i + h
                            mm(nc, ps[:, h * 512:(h + 1) * 512],
                               kt[:, t * 128:(t + 1) * 128], qs,
                               start=True, stop=True)
                        et = etp.tile([128, 1024], F16, tag="et",
                                      name=f"et{j}_{i}")
                        nc.scalar.activation(
                            et[:], ps[:], mybir.ActivationFunctionType.Exp)

                        def pv_pair(i=i, et=et, first=first, last=last):
                            mm(nc, pv[:], vaug[:, 2 * i, :], et[:, 0:512],
                               start=first, stop=False)
                            mm(nc, pv[:], vaug[:, 2 * i + 1, :],
                               et[:, 512:1024], start=False, stop=last)
                    pend.append(pv_pair)
                    if len(pend) > 2:
                        pend.pop(0)()

                def emit_out(j=j, pv=pv):
                    nc.vector.tensor_copy(out_sb[:, j * 512:(j + 1) * 512],
                                          pv[:])
                pend.append(emit_out)

            # ---- schedule ----
            # All loads 512-col, issue queues: Activation (idle until the
            # first exp) takes the two most urgent, SP takes x_q, Pool takes
            # the rest; per-queue order matches need order. PE emission order
            # tracks the expected DMA landing order (in-order engine: a
            # stalled instruction blocks later ready ones).
            nc.gpsimd.dma_start(out=wts[:], in_=wts_d[:])
            load_cols(nc.sync, xkT, xkt_d, 0, 512)
            load_cols(nc.scalar, xqT, xqt_d, 0, 512)
            load_cols(nc.gpsimd, xvT, xvt_d, 0, 512)
            nc.sync.dma_start(out=cmask[:], in_=cm_d[:])
            load_cols(nc.scalar, xkT, xkt_d, 512, 1024)
            load_cols(nc.sync, xqT, xqt_d, 512, 1024)
            load_cols(nc.gpsimd, xvT, xvt_d, 512, 1024)
            load_cols(nc.sync, xqT, xqt_d, 1024, 1536)
            load_cols(nc.gpsimd, xkT, xkt_d, 1024, 1536)
            load_cols(nc.sync, xqT, xqt_d, 1536, 2048)
            load_cols(nc.gpsimd, xvT, xvt_d, 1024, 1536)
            load_cols(nc.sync, xqT, xqt_d, 2048, 2560)
            load_cols(nc.gpsimd, xkT, xkt_d, 1536, 2048)
            load_cols(nc.sync, xqT, xqt_d, 2560, 3072)
            load_cols(nc.gpsimd, xvT, xvt_d, 1536, 2048)
            load_cols(nc.sync, xqT, xqt_d, 3072, 3584)
            load_cols(nc.sync, xqT, xqt_d, 3584, 4096)

            proj_chunk(kt, xkT, w_sb["wk"], 0, "k")
            proj_chunk(qt, xqT, w_sb["wq"], 0, "q")
            attn_chunk(0)       # pair 0 scores; pv00 pends on v_pair(0)
            warmN(6)
            v_pair(0)
            v_pair(1)
            warmN(2)
            proj_chunk(qt, xqT, w_sb["wq"], 1, "q")
            attn_chunk(1)
            warmN(3)
            proj_chunk(kt, xkT, w_sb["wk"], 1, "k")
            proj_chunk(qt, xqT, w_sb["wq"], 2, "q")
            attn_chunk(2)
            v_pair(2)
            v_pair(3)
            warmN(5)
            proj_chunk(qt, xqT, w_sb["wq"], 3, "q")
            attn_chunk(3)
            warmN(3)
            proj_chunk(kt, xkT, w_sb["wk"], 2, "k")
            proj_chunk(qt, xqT, w_sb["wq"], 4, "q")
            attn_chunk(4)
            v_pair(4)
            v_pair(5)
            nc.sync.dma_start(out=pvt[:, 0:2048], in_=out_sb[:, 0:2048])
            proj_chunk(qt, xqT, w_sb["wq"], 5, "q")
            attn_chunk(5)
            proj_chunk(kt, xkT, w_sb["wk"], 3, "k")
            proj_chunk(qt, xqT, w_sb["wq"], 6, "q")
            attn_chunk(6)
            v_pair(6)
            v_pair(7)
            proj_chunk(qt, xqT, w_sb["wq"], 7, "q")
            attn_chunk(7)
            nc.sync.dma_start(out=pvt[:, 2048:3584],
                              in_=out_sb[:, 2048:3584])
            flush_pend()
            nc.sync.dma_start(out=pvt[:, 3584:4096],
                              in_=out_sb[:, 3584:4096])
    nc.compile()
    return nc


def _prep_w(w, scale=1.0):
    # [512, 64] -> [128, 4*64]: (p, dt*64+e) holds W[dt*128 + p, e] so the
    # lhsT slice [:, dt, :] matches X^T d-block dt.
    return (np.asarray(w, np.float64) * scale).reshape(
        4, 128, D_OUT).transpose(1, 0, 2).reshape(128, 256).astype(np.float16)


def kernel(inputs_for_keys, inputs_for_values, inputs_for_queries, WK, WV, WQ):
    xk_f = np.asarray(inputs_for_keys, np.float16)
    xv_f = np.asarray(inputs_for_values, np.float16)
    xq_f = np.asarray(inputs_for_queries, np.float16)
    wkp = _prep_w(WK)
    wvp = _prep_w(WV)
    wqp = _prep_w(WQ, scale=1.0 / np.sqrt(np.float64(S)))
    wts = np.ascontiguousarray(np.concatenate([wqp, wkp, wvp], axis=1))

    if "nc" not in _CACHE:
        _CACHE["nc"] = build_nc()
    nc = _CACHE["nc"]

    # diag-pair mask, 768 kept columns:
    #   [k, 0:512]  : tile 2j  -> 1 if q >= k + 128*hk
    #   [k, 512:768]: tile 2j+1, q-half 1 -> 1 if q >= k + 256 + 128*hk
    kk = np.arange(128)[:, None]
    qq = np.arange(512)[None, :]
    cms = []
    for hk in range(2):
        m0 = (qq >= kk + 128 * hk).astype(np.float16)
        m1 = (qq >= kk + 256 + 128 * hk).astype(np.float16)[:, 256:512]
        cms.append(np.ascontiguousarray(np.concatenate([m0, m1], axis=1)))

    in_maps = []
    xqt_b = [np.ascontiguousarray(xq_f[b].T) for b in range(B)]
    for c in range(N_CORES):
        b, hk = c // 2, c % 2
        xk_g = xk_f[b].reshape(S // 128, 128, D_IN)[hk::2].reshape(SK, D_IN)
        xv_g = xv_f[b].reshape(S // 128, 128, D_IN)[hk::2].reshape(SK, D_IN)
        in_maps.append({
            "xqt": xqt_b[b],
            "xkt": np.ascontiguousarray(xk_g.T),
            "xvt": np.ascontiguousarray(xv_g.T),
            "wts": wts,
            "cmask": cms[hk],
        })

    _CACHE["in_maps"] = in_maps
    res = run_bass_kernel_spmd(nc, in_maps, core_ids=list(range(N_CORES)))
    out = np.empty((B, S, D_OUT), np.float32)
    for b in range(B):
        p = (res.results[2 * b]["pvt"].astype(np.float32)
             + res.results[2 * b + 1]["pvt"].astype(np.float32))
        out[b] = (p[0:D_OUT, :] / p[D_OUT:D_OUT + 1, :]).T
    return out


# revision 52
# speedup vs baseline: 1.0079x; 1.0079x over previous
"""Causal attention head kernel for Trainium2, 8 NeuronCores.

Problem: B=4, S=4096, D_IN=512, D_OUT=64, f32, causal, scale=1/sqrt(S).

Sharding: core c -> (batch b = c//2, k-shard hk = c%2). Each core handles ALL
queries of its batch but only the k-tiles (of 128 rows) with tile_index % 2 ==
hk, producing partial (numerator | denominator) sums; the host combines the
two k-shards. SPMD: causality differences between the two k-shards live in a
per-core mask input and in the per-core gather of X_k/X_v rows.

fp16 datapath (inputs cast on host: halves DMA, ~10-bit mantissa keeps the
f32r-class accuracy). q-chunk-outer attention with PV accumulated in PSUM
across all k-pairs of the chunk; softmax denominator rides as a ones-column
(65th row) of V. PE stream is software-pipelined (scores of the next pair are
emitted before PV of the previous one) so exp latency stays off the PE
critical path; warm-up matmuls at t=0 ramp the PE clock before real work
arrives. The diagonal pair's second k-tile only covers the upper q-half
(causally valid for both shards); the diag pair runs last per chunk (its V
tiles are the freshest load) except in the final chunk where it runs first
to keep exp+mask off the tail, and the final pair is split into single-tile
halves. Loads are 512-col chunks, need-order sorted, issued from the SP /
Activation / Pool sequencers (a DMA holds its issuing SEQ for ~2.2us fixed
overhead plus the transfer, and all transfers serialize on the DMA engines);
output staging goes PSUM -> SBUF on DVE (gpsimd cannot access PSUM), with
three batched output DMAs on SP.
"""

import os

os.environ.setdefault("JAX_PLATFORMS", "cpu")

import numpy as np

import concourse.bass as bass
import concourse.bacc as bacc
import concourse.mybir as mybir
from concourse import tile
from concourse.bass_utils import run_bass_kernel_spmd

F32 = mybir.dt.float32
F16 = mybir.dt.float16

B, S, D_IN, D_OUT = 4, 4096, 512, 64
SK = S // 2          # per-core k rows (interleaved 128-tiles)
N_KT = SK // 128     # 16 local k-tiles
N_QC = S // 512      # 8 q-chunks of 512
N_CORES = 8

_CACHE = {}


def mm(nc, out, lhsT, rhs, start, stop):
    nc.tensor.matmul(out, lhsT, rhs, start=start, stop=stop)


def build_nc():
    nc = bacc.Bacc(trn_type="TRN2", target_bir_lowering=False, debug=False)

    xqt_d = nc.dram_tensor("xqt", [D_IN, S], F16, kind="ExternalInput").ap()
    xkt_d = nc.dram_tensor("xkt", [D_IN, SK], F16, kind="ExternalInput").ap()
    xvt_d = nc.dram_tensor("xvt", [D_IN, SK], F16, kind="ExternalInput").ap()
    wts_d = nc.dram_tensor("wts", [128, 768], F16, kind="ExternalInput").ap()
    cm_d = nc.dram_tensor("cmask", [128, 768], F16, kind="ExternalInput").ap()
    pvt = nc.dram_tensor("pvt", [D_OUT + 1, S], F16, kind="ExternalOutput").ap()

    with tile.TileContext(nc) as tc:
        with (
            tc.tile_pool(name="persist", bufs=1) as pp,
            tc.tile_pool(name="et", bufs=5) as etp,
            tc.tile_pool(name="ps_s", bufs=2, space="PSUM") as ps_s,
            tc.tile_pool(name="ps_pv", bufs=2, space="PSUM") as ps_pv,
            tc.tile_pool(name="ps_pr", bufs=2, space="PSUM") as ps_pr,
        ):
            # ---- persistent SBUF tiles ----
            xqT = pp.tile([128, 4, S], F16, tag="xqT", name="xqT")
            xkT = pp.tile([128, 4, SK], F16, tag="xkT", name="xkT")
            xvT = pp.tile([128, 4, SK], F16, tag="xvT", name="xvT")
            qt = pp.tile([64, S], F16, tag="qt", name="qt")
            kt = pp.tile([64, SK], F16, tag="kt", name="kt")
            vaug = pp.tile([128, N_KT, D_OUT + 1], F16, tag="vaug", name="vaug")
            wts = pp.tile([128, 768], F16, tag="wts", name="wts")
            cmask = pp.tile([128, 768], F16, tag="cmask", name="cmask")
            warm = pp.tile([128, 512], F16, tag="warm", name="warm")
            out_sb = pp.tile([D_OUT + 1, S], F16, tag="out_sb", name="out_sb")

            # PE warm-up: ramp the tensor clock while loads are in flight.
            # The values feed an unread PSUM; only the busy time matters.
            # The prelude reads the (not-yet-loaded) cmask tile so the PE can
            # start at t~0 with no memset ahead of it; the cmask DMA simply
            # picks up a WAR dependency on the prelude. Fillers use the
            # memset warm tile (their WAR would otherwise delay the mask).
            wps = ps_pr.tile([128, 512], F32, tag="ps_pr", name="warm_ps")

            def warmN(n, src_tile=None):
                t = warm if src_tile is None else src_tile
                for _ in range(n):
                    mm(nc, wps[0:128, 0:256], t[:, 0:128], t[:, 0:256],
                       start=True, stop=True)

            warmN(8, cmask)
            nc.vector.memset(warm[:, 0:256], 0.0)
            warmN(11)
            w_sb = {nm: wts[:, 256 * i:256 * (i + 1)].rearrange(
                        "p (dt e) -> p dt e", dt=4)
                    for i, nm in enumerate(("wq", "wk", "wv"))}
            # ones column -> softmax denominator via the PV matmul
            nc.vector.memset(vaug[:, :, D_OUT], 1.0)

            def load_cols(eng, dst_tile, src_ap, c0, c1):
                sl = slice(c0, c1)
                src = src_ap.rearrange("(db p) c -> p db c", p=128)
                eng.dma_start(out=dst_tile[:, :, sl], in_=src[:, :, sl])

            def proj_chunk(dst, xT, w, c, nm):
                sl = slice(c * 512, (c + 1) * 512)
                ps = ps_pr.tile([128, 512], F32, tag="ps_pr", name=f"pp_{nm}{c}")
                for dt in range(4):
                    mm(nc, ps[0:64, :], w[:, dt, :], xT[:, dt, sl],
                       start=(dt == 0), stop=(dt == 3))
                nc.vector.tensor_copy(dst[:, sl], ps[0:64, :])

            def v_pair(i):
                # V natural for s-tiles 2i, 2i+1: [128s,64] per tile
                for t in (2 * i, 2 * i + 1):
                    ps = ps_pr.tile([128, 512], F32, tag="ps_pr", name=f"pv_{t}")
                    for dt in range(4):
                        mm(nc, ps[:, 0:D_OUT],
                           xvT[:, dt, t * 128:(t + 1) * 128],
                           w_sb["wv"][:, dt, :],
                           start=(dt == 0), stop=(dt == 3))
                    nc.vector.tensor_copy(vaug[:, t, 0:D_OUT], ps[:, 0:D_OUT])

            # deferred PV pairs: flushed after the next PE work is emitted so
            # the PE never stalls waiting for the exp feeding it
            pend = []

            def flush_pend():
                while pend:
                    pend.pop(0)()

            def attn_chunk(j):
                # diagonal pair last (its V tiles are the freshest load)
                # except in the final chunk, where diag-first keeps the
                # exp+mask chain off the critical tail; PV accumulates in one
                # PSUM tile across all pairs
                qs = qt[:, j * 512:(j + 1) * 512]
                pv = ps_pv.tile([65, 512], F32, tag="ps_pv", name=f"pvp{j}")
                order = [j] + list(range(j)) if j == 7 else \
                    list(range(j)) + [j]
                for n, i in enumerate(order):
                    first, last = (n == 0), (n == len(order) - 1)
                    if i == j:
                        # diag pair: tile 2j over all q, tile 2j+1 only the
                        # upper q-half (lower half fully masked on both
                        # shards); mask input covers the 768 kept columns
                        ps = ps_d = ps_s.tile([128, 768], F32, tag="ps_s",
                                              name=f"st{j}_d")
                        mm(nc, ps[:, 0:512], kt[:, 2 * j * 128:(2 * j + 1) * 128],
                           qs, start=True, stop=True)
                        mm(nc, ps[:, 512:768],
                           kt[:, (2 * j + 1) * 128:(2 * j + 2) * 128],
                           qs[:, 256:512], start=True, stop=True)
                        et = etp.tile([128, 768], F16, tag="et", name=f"et{j}_d")
                        nc.scalar.activation(
                            et[:], ps[:], mybir.ActivationFunctionType.Exp)
                        nc.vector.tensor_mul(et[:], et[:], cmask[:])

                        def pv_pair(i=i, et=et, first=first, last=last):
                            mm(nc, pv[:], vaug[:, 2 * i, :], et[:, 0:512],
                               start=first, stop=False)
                            mm(nc, pv[:, 256:512], vaug[:, 2 * i + 1, :],
                               et[:, 512:768], start=False, stop=last)
                    elif j == 7 and last:
                        # final pair of the final chunk: two single-tile
                        # halves so the last exp covers only 512 columns
                        for h in range(2):
                            t = 2 * i + h
                            ps = ps_s.tile([128, 512], F32, tag="ps_s",
                                           name=f"st7_{i}{h}")
                            mm(nc, ps[:], kt[:, t * 128:(t + 1) * 128], qs,
                               start=True, stop=True)
                            et = etp.tile([128, 512], F16, tag="et",
                                          name=f"et7_{i}{h}")
                            nc.scalar.activation(
                                et[:], ps[:],
                                mybir.ActivationFunctionType.Exp)

                            def pv_half(t=t, et=et, h=h):
                                mm(nc, pv[:], vaug[:, t, :], et[:],
                                   start=False, stop=(h == 1))
                            pend.append(pv_half)
                            if len(pend) > 2:
                                pend.pop(0)()
                        continue
                    else:
                        ps = ps_s.tile([128, 1024], F32, tag="ps_s",
                                       name=f"st{j}_{i}")
                        for h in range(2):
                            t = 2 * i + h
                            mm(nc, ps[:, h * 512:(h + 1) * 512],
                               kt[:, t * 128:(t + 1) * 128], qs,
                               start=True, stop=True)
                        et = etp.tile([128, 1024], F16, tag="et",
                                      name=f"et{j}_{i}")
                        nc.scalar.activation(
                            et[:], ps[:], mybir.ActivationFunctionType.Exp)

                        def pv_pair(i=i, et=et, first=first, last=last):
                            mm(nc, pv[:], vaug[:, 2 * i, :], et[:, 0:512],
                               start=first, stop=False)
                            mm(nc, pv[:], vaug[:, 2 * i + 1, :],
                               et[:, 512:1024], start=False, stop=last)
                    pend.append(pv_pair)
                    if len(pend) > 2:
                        pend.pop(0)()

                def emit_out(j=j, pv=pv):
                    nc.vector.tensor_copy(out_sb[:, j * 512:(j + 1) * 512],
                                          pv[:])
                pend.append(emit_out)

            # ---- schedule ----
            # All loads 512-col, issue queues: Activation (idle until the
            # first exp) takes the two most urgent, SP takes x_q, Pool takes
            # the rest; per-queue order matches need order. PE emission order
            # tracks the expected DMA landing order (in-order engine: a
            # stalled instruction blocks later ready ones).
            nc.gpsimd.dma_start(out=wts[:], in_=wts_d[:])
            load_cols(nc.sync, xkT, xkt_d, 0, 512)
            load_cols(nc.scalar, xqT, xqt_d, 0, 512)
            load_cols(nc.gpsimd, xvT, xvt_d, 0, 512)
            nc.sync.dma_start(out=cmask[:], in_=cm_d[:])
            load_cols(nc.scalar, xkT, xkt_d, 512, 1024)
            load_cols(nc.sync, xqT, xqt_d, 512, 1024)
            load_cols(nc.gpsimd, xvT, xvt_d, 512, 1024)
            load_cols(nc.sync, xqT, xqt_d, 1024, 1536)
            load_cols(nc.gpsimd, xkT, xkt_d, 1024, 1536)
            load_cols(nc.sync, xqT, xqt_d, 1536, 2048)
            load_cols(nc.gpsimd, xvT, xvt_d, 1024, 1536)
            load_cols(nc.sync, xqT, xqt_d, 2048, 2560)
            load_cols(nc.gpsimd, xkT, xkt_d, 1536, 2048)
            load_cols(nc.sync, xqT, xqt_d, 2560, 3072)
            load_cols(nc.gpsimd, xvT, xvt_d, 1536, 2048)
            load_cols(nc.sync, xqT, xqt_d, 3072, 3584)
            load_cols(nc.sync, xqT, xqt_d, 3584, 4096)

            proj_chunk(kt, xkT, w_sb["wk"], 0, "k")
            proj_chunk(qt, xqT, w_sb["wq"], 0, "q")
            attn_chunk(0)       # pair 0 scores; pv00 pends on v_pair(0)
            warmN(6)
            v_pair(0)
            v_pair(1)
            warmN(2)
            proj_chunk(qt, xqT, w_sb["wq"], 1, "q")
            attn_chunk(1)
            warmN(3)
            proj_chunk(kt, xkT, w_sb["wk"], 1, "k")
            proj_chunk(qt, xqT, w_sb["wq"], 2, "q")
            attn_chunk(2)
            v_pair(2)
            v_pair(3)
            warmN(5)
            proj_chunk(qt, xqT, w_sb["wq"], 3, "q")
            attn_chunk(3)
            warmN(3)
            proj_chunk(kt, xkT, w_sb["wk"], 2, "k")
            proj_chunk(qt, xqT, w_sb["wq"], 4, "q")
            attn_chunk(4)
            v_pair(4)
            v_pair(5)
            nc.sync.dma_start(out=pvt[:, 0:2048], in_=out_sb[:, 0:2048])
            proj_chunk(qt, xqT, w_sb["wq"], 5, "q")
            attn_chunk(5)
            proj_chunk(kt, xkT, w_sb["wk"], 3, "k")
            proj_chunk(qt, xqT, w_sb["wq"], 6, "q")
            attn_chunk(6)
            v_pair(6)
            v_pair(7)
            proj_chunk(qt, xqT, w_sb["wq"], 7, "q")
            attn_chunk(7)
            nc.sync.dma_start(out=pvt[:, 2048:3584],
                              in_=out_sb[:, 2048:3584])
            flush_pend()
            nc.sync.dma_start(out=pvt[:, 3584:4096],
                              in_=out_sb[:, 3584:4096])
    nc.compile()
    return nc


def _prep_w(w, scale=1.0):
    # [512, 64] -> [128, 4*64]: (p, dt*64+e) holds W[dt*128 + p, e] so the
    # lhsT slice [:, dt, :] matches X^T d-block dt.
    return (np.asarray(w, np.float64) * scale).reshape(
        4, 128, D_OUT).transpose(1, 0, 2).reshape(128, 256).astype(np.float16)


def kernel(inputs_for_keys, inputs_for_values, inputs_for_queries, WK, WV, WQ):
    xk_f = np.asarray(inputs_for_keys, np.float16)
    xv_f = np.asarray(inputs_for_values, np.float16)
    xq_f = np.asarray(inputs_for_queries, np.float16)
    wkp = _prep_w(WK)
    wvp = _prep_w(WV)
    wqp = _prep_w(WQ, scale=1.0 / np.sqrt(np.float64(S)))
    wts = np.ascontiguousarray(np.concatenate([wqp, wkp, wvp], axis=1))

    if "nc" not in _CACHE:
        _CACHE["nc"] = build_nc()
    nc = _CACHE["nc"]

    # diag-pair mask, 768 kept columns:
    #   [k, 0:512]  : tile 2j  -> 1 if q >= k + 128*hk
    #   [k, 512:768]: tile 2j+1, q-half 1 -> 1 if q >= k + 256 + 128*hk
    kk = np.arange(128)[:, None]
    qq = np.arange(512)[None, :]
    cms = []
    for hk in range(2):
        m0 = (qq >= kk + 128 * hk).astype(np.float16)
        m1 = (qq >= kk + 256 + 128 * hk).astype(np.float16)[:, 256:512]
        cms.append(np.ascontiguousarray(np.concatenate([m0, m1], axis=1)))

    in_maps = []
    xqt_b = [np.ascontiguousarray(xq_f[b].T) for b in range(B)]
    for c in range(N_CORES):
        b, hk = c // 2, c % 2
        xk_g = xk_f[b].reshape(S // 128, 128, D_IN)[hk::2].reshape(SK, D_IN)
        xv_g = xv_f[b].reshape(S // 128, 128, D_IN)[hk::2].reshape(SK, D_IN)
        in_maps.append({
            "xqt": xqt_b[b],
            "xkt": np.ascontiguousarray(xk_g.T),
            "xvt": np.ascontiguousarray(xv_g.T),
            "wts": wts,
            "cmask": cms[hk],
        })

    _CACHE["in_maps"] = in_maps
    res = run_bass_kernel_spmd(nc, in_maps, core_ids=list(range(N_CORES)))
    out = np.empty((B, S, D_OUT), np.float32)
    for b in range(B):
        p = (res.results[2 * b]["pvt"].astype(np.float32)
             + res.results[2 * b + 1]["pvt"].astype(np.float32))
        out[b] = (p[0:D_OUT, :] / p[D_OUT:D_OUT + 1, :]).T
    return out
